# revision 3
# baseline (speedup 1.0000x reference)
"""Trainium2 Bass kernel for BasicQuadRGBModel (quad-Bayer demosaic CNN).

v2 layout (engine APs need partition base in {0,32,64,96}; DMA is exempt):
  - im2col buffers R [120p, 10 rows, 64 win]: main block xa=1..8 at partitions
    (xa-1)*12+ci = [0:96); xa=0 strip at [96:108); xa=9 strip at [108:120).
    PSUM eviction is then a base0->base0 relu copy; strips are SBUF->SBUF DMAs.
  - grb/d buffers [20p]: main (xa-1)*2+c at [0:16); strips [16:18),[18:20).
  - layer-0 im2col r0 [128p] host-built: ky0 block [0:40), ky1-other [40:60),
    ky1-rb [64:84) (aligned: feeds d_buf copies), ky2 block [84:124).
  - conv = 3 accumulating matmuls/layer (K=120, M=96, N=512 = 8 rows x 64 win);
    softmax/green/chroma folded into small matmuls; float32r for full PE rate.
  - host does layer-0 im2col and the final 2x2 pixel-shuffle.
"""

import sys

sys.path.insert(0, "/opt/trn_rl_repo")

import numpy as np

import concourse.bass as bass
import concourse.mybir as mybir
import concourse.tile as tile
from concourse import bacc
from concourse.bass_utils import run_bass_kernel_spmd

N_CORES = 8
B_PC = 2
H = W = 512
NW = 64
NSLAB = 64
CH = 12
F32 = mybir.dt.float32
F32R = mybir.dt.float32r
USE_F32R = True
OUTPUT_NAMES = ["out_cp", "out_g"]


def _rbloc(xa, c):
    if xa == 0:
        return 16 + c
    if xa == 9:
        return 18 + c
    return (xa - 1) * 2 + c


def _rloc(xa, ci):
    if xa == 0:
        return 96 + ci
    if xa == 9:
        return 108 + ci
    return (xa - 1) * 12 + ci


def _r0loc(ky, ci, xa):
    if ky == 0:
        if ci == 0:
            return xa
        if ci == 3:
            return 10 + xa
        return 20 + _rbloc(xa, ci - 1)
    if ky == 1:
        if ci == 0:
            return 40 + xa
        if ci == 3:
            return 50 + xa
        return 64 + _rbloc(xa, ci - 1)
    if ci == 0:
        return 84 + xa
    if ci == 3:
        return 94 + xa
    return 104 + _rbloc(xa, ci - 1)


def build_r0(mosaic):
    B = mosaic.shape[0]
    mp = np.zeros((B, 4, H + 2, W + 2), np.float32)
    mp[:, :, 1 : H + 1, 1 : W + 1] = mosaic
    r0 = np.zeros((B, 128, H, NW), np.float32)
    for ky in range(3):
        for ci in range(4):
            for xa in range(10):
                r0[:, _r0loc(ky, ci, xa)] = mp[:, ci, ky : ky + H, xa : xa + 8 * NW : 8]
    return r0


def build_w_l0(wt):
    W_ = np.zeros((128, 96), np.float32)
    for ky in range(3):
        for ci in range(4):
            for xa in range(10):
                for xo in range(8):
                    kx = xa - xo
                    if 0 <= kx <= 2:
                        for co in range(CH):
                            W_[_r0loc(ky, ci, xa), xo * 12 + co] = wt[co, ci, ky, kx]
    return W_


def build_w_int(wt):
    W_ = np.zeros((3, 120, 96), np.float32)
    for ky in range(3):
        for xa in range(10):
            for xo in range(8):
                kx = xa - xo
                if 0 <= kx <= 2:
                    k = _rloc(xa, 0)
                    W_[ky, k : k + 12, xo * 12 : xo * 12 + 12] = wt[:, :, ky, kx].T
    return W_


def build_w_sums():
    wse = np.zeros((96, 8), np.float32)
    wsep = np.zeros((96, 16), np.float32)
    wbc = np.zeros((8, 16), np.float32)
    for xo in range(8):
        for co in range(CH):
            wse[xo * 12 + co, xo] = 1.0
            wsep[xo * 12 + co, xo * 2 + (co >= 6)] = 1.0
        wbc[xo, xo * 2 : xo * 2 + 2] = 1.0
    return wse, wsep, wbc


def build_w_chroma(cw0):
    wchk = np.zeros((3, 20, 48), np.float32)
    for ky in range(3):
        for xa in range(10):
            for xo in range(8):
                kx = xa - xo
                if 0 <= kx <= 2:
                    for co in range(6):
                        for d in range(2):
                            wchk[ky, _rbloc(xa, d), xo * 6 + co] = cw0[co, d, ky, kx]
    # green_add = [m0, g1, m3, m0, g0, m3]; g0 = m1 - d0, g1 = m2 - d1
    for xo in range(8):
        wchk[1, _rbloc(xo + 1, 1), xo * 6 + 1] += -1.0
        wchk[1, _rbloc(xo + 1, 0), xo * 6 + 4] += -1.0
    wchm = np.zeros((128, 48), np.float32)
    for xo in range(8):
        xa = xo + 1
        wchm[_r0loc(1, 0, xa), xo * 6 + 0] = 1.0
        wchm[_r0loc(1, 0, xa), xo * 6 + 3] = 1.0
        wchm[_r0loc(1, 3, xa), xo * 6 + 2] = 1.0
        wchm[_r0loc(1, 3, xa), xo * 6 + 5] = 1.0
        wchm[_r0loc(1, 2, xa), xo * 6 + 1] = 1.0
        wchm[_r0loc(1, 1, xa), xo * 6 + 4] = 1.0
    return wchk, wchm


def assemble_output(mosaic, cp_dev, g_dev):
    B = mosaic.shape[0]
    cp = cp_dev.reshape(B, 8, 6, H, NW).transpose(0, 2, 3, 4, 1).reshape(B, 6, H, W)
    g = g_dev.reshape(B, 8, 2, H, NW).transpose(0, 2, 3, 4, 1).reshape(B, 2, H, W)
    m = mosaic
    out = np.empty((B, 3, 2 * H, 2 * W), np.float32)
    out[:, 0, 0::2, 0::2] = cp[:, 0]
    out[:, 0, 0::2, 1::2] = m[:, 1]
    out[:, 0, 1::2, 0::2] = cp[:, 1]
    out[:, 0, 1::2, 1::2] = cp[:, 2]
    out[:, 1, 0::2, 0::2] = m[:, 0]
    out[:, 1, 0::2, 1::2] = g[:, 0]
    out[:, 1, 1::2, 0::2] = g[:, 1]
    out[:, 1, 1::2, 1::2] = m[:, 3]
    out[:, 2, 0::2, 0::2] = cp[:, 3]
    out[:, 2, 0::2, 1::2] = cp[:, 4]
    out[:, 2, 1::2, 0::2] = m[:, 2]
    out[:, 2, 1::2, 1::2] = cp[:, 5]
    return out


def _mm_dt(ap):
    return ap.bitcast(F32R) if USE_F32R else ap


# column offsets inside the packed [128, 1576] stationary tensor
_WOFF = {"wf0": 0, "ww0": 96, "wf1": 192, "wf2": 480, "ww1": 768, "ww2": 1056,
         "wse": 1344, "wsep": 1352, "wbc": 1368, "wchk": 1384, "wchm": 1528}
_WCOLS = 1576


def pack_stationaries(st):
    wp = np.zeros((128, _WCOLS), np.float32)
    wp[:, 0:96] = st["wf0"]
    wp[:, 96:192] = st["ww0"]
    for nm in ("wf1", "wf2", "ww1", "ww2"):
        o = _WOFF[nm]
        for ky in range(3):
            wp[0:120, o + 96 * ky : o + 96 * (ky + 1)] = st[nm][ky]
    wp[0:96, 1344:1352] = st["wse"]
    wp[0:96, 1352:1368] = st["wsep"]
    wp[0:8, 1368:1384] = st["wbc"]
    for ky in range(3):
        wp[0:20, 1384 + 48 * ky : 1384 + 48 * (ky + 1)] = st["wchk"][ky]
    wp[:, 1528:1576] = st["wchm"]
    return wp


_W_SHAPES = [
    ("wf0", [128, 96]),
    ("ww0", [128, 96]),
    ("wf1", [120, 3, 96]),
    ("wf2", [120, 3, 96]),
    ("ww1", [120, 3, 96]),
    ("ww2", [120, 3, 96]),
    ("wse", [96, 8]),
    ("wsep", [96, 16]),
    ("wbc", [8, 16]),
    ("wchk", [20, 3, 48]),
    ("wchm", [128, 48]),
]


def build_program():
    from contextlib import ExitStack

    nc = bacc.Bacc(
        "TRN2", target_bir_lowering=False, debug=False, num_devices=N_CORES
    )
    r0 = nc.declare_dram_parameter("r0", [B_PC, 128, H, NW], F32, isOutput=False)
    wpack = nc.declare_dram_parameter("wpack", [128, _WCOLS], F32, isOutput=False)
    out_cp = nc.declare_dram_parameter("out_cp", [B_PC, 48, H, NW], F32, isOutput=True)
    out_g = nc.declare_dram_parameter("out_g", [B_PC, 16, H, NW], F32, isOutput=True)

    Relu = mybir.ActivationFunctionType.Relu
    Exp = mybir.ActivationFunctionType.Exp
    Copy = mybir.ActivationFunctionType.Copy
    NSTEPS = B_PC * NSLAB

    with tile.TileContext(nc) as tc, ExitStack() as ctx:
        const = ctx.enter_context(tc.tile_pool(name="const", bufs=1))
        r0pool = ctx.enter_context(tc.tile_pool(name="r0pool", bufs=6))
        p_rf1 = ctx.enter_context(tc.tile_pool(name="rf1", bufs=4))
        p_rw1 = ctx.enter_context(tc.tile_pool(name="rw1", bufs=4))
        p_rf2 = ctx.enter_context(tc.tile_pool(name="rf2", bufs=4))
        p_rw2 = ctx.enter_context(tc.tile_pool(name="rw2", bufs=4))
        p_grb = ctx.enter_context(tc.tile_pool(name="grb", bufs=4))
        p_d = ctx.enter_context(tc.tile_pool(name="dbuf", bufs=2))
        p_act = ctx.enter_context(tc.tile_pool(name="acts", bufs=3))
        p_stg = ctx.enter_context(tc.tile_pool(name="stg", bufs=3))
        ps_mm = ctx.enter_context(tc.tile_pool(name="psmm", bufs=4, space="PSUM"))
        ps_sm = ctx.enter_context(tc.tile_pool(name="pssm", bufs=2, space="PSUM"))
        ps_cp = ctx.enter_context(tc.tile_pool(name="pscp", bufs=2, space="PSUM"))

        WC = const.tile([128, _WCOLS], F32, tag="wpack_sb", name="wpack_sb")
        nc.sync.dma_start(out=WC[:], in_=wpack[:])
        sb = {
            "wf0": WC[:, 0:96],
            "ww0": WC[:, 96:192],
            "wse": WC[0:96, 1344:1352],
            "wsep": WC[0:96, 1352:1368],
            "wbc": WC[0:8, 1368:1384],
            "wchm": WC[:, 1528:1576],
        }

        def wky(nm, ky):
            o = _WOFF[nm]
            if nm == "wchk":
                return WC[0:20, o + 48 * ky : o + 48 * (ky + 1)]
            return WC[0:120, o + 96 * ky : o + 96 * (ky + 1)]

        r0s, rf1, rw1, rf2, rw2, grb = {}, {}, {}, {}, {}, {}

        def get_rbuf(pool, dct, s):
            if s in dct or not (0 <= s < NSTEPS):
                return dct.get(s)
            t = pool.tile([120, 10, NW], F32)
            dct[s] = t
            sl = s % NSLAB
            if sl == 0:
                nc.vector.memset(t[:, 0:1, :], 0.0)
            if sl == NSLAB - 1:
                nc.vector.memset(t[:, 9:10, :], 0.0)
            nc.vector.memset(t[96:120, :, 0:1], 0.0)
            nc.vector.memset(t[96:120, :, 63:64], 0.0)
            return t

        def get_grb(s):
            if s in grb or not (0 <= s < NSTEPS):
                return grb.get(s)
            t = p_grb.tile([20, 10, NW], F32, name="g")
            grb[s] = t
            sl = s % NSLAB
            if sl == 0:
                nc.vector.memset(t[:, 0:1, :], 0.0)
            if sl == NSLAB - 1:
                nc.vector.memset(t[:, 9:10, :], 0.0)
            nc.vector.memset(t[:, :, 0:1], 0.0)
            nc.vector.memset(t[:, :, 63:64], 0.0)
            return t

        def conv_int(nm, rbuf):
            ps = ps_mm.tile([96, 8, NW], F32, tag="mm96", name="psc")
            for ky in range(3):
                nc.tensor.matmul(
                    ps[:],
                    _mm_dt(wky(nm, ky)),
                    _mm_dt(rbuf[:, ky : ky + 8, :]),
                    start=(ky == 0),
                    stop=(ky == 2),
                )
            return ps

        def evict(ps, dct, s):
            sl = s % NSLAB
            nc.scalar.activation(out=dct[s][0:96, 1:9, :], in_=ps[:], func=Relu)
            if sl < NSLAB - 1:
                nc.scalar.activation(
                    out=dct[s + 1][0:96, 0:1, :], in_=ps[:, 7:8, :], func=Relu
                )
            if sl > 0:
                nc.scalar.activation(
                    out=dct[s - 1][0:96, 9:10, :], in_=ps[:, 0:1, :], func=Relu
                )

        def strips(t):
            nc.sync.dma_start(out=t[96:108, :, 1:NW], in_=t[84:96, :, 0 : NW - 1])
            nc.sync.dma_start(out=t[108:120, :, 0 : NW - 1], in_=t[0:12, :, 1:NW])

        for T in range(NSTEPS + 3):
            s0 = T
            if 0 <= s0 < NSTEPS:
                img, sl = divmod(s0, NSLAB)
                y0 = sl * 8
                rt = r0pool.tile([128, 8, NW], F32, name="rt")
                r0s[s0] = rt
                nc.sync.dma_start(out=rt[:], in_=r0[img, :, y0 : y0 + 8, :])
                get_rbuf(p_rf1, rf1, s0)
                get_rbuf(p_rf1, rf1, s0 + 1)
                get_rbuf(p_rw1, rw1, s0)
                get_rbuf(p_rw1, rw1, s0 + 1)
                psf = ps_mm.tile([96, 8, NW], F32, tag="mm96", name="psf0")
                nc.tensor.matmul(
                    psf[:], _mm_dt(sb["wf0"]), _mm_dt(rt[:]), start=True, stop=True
                )
                evict(psf, rf1, s0)
                psw = ps_mm.tile([96, 8, NW], F32, tag="mm96", name="psw0")
                nc.tensor.matmul(
                    psw[:], _mm_dt(sb["ww0"]), _mm_dt(rt[:]), start=True, stop=True
                )
                evict(psw, rw1, s0)

            s1 = T - 1
            if 0 <= s1 < NSTEPS:
                strips(rf1[s1])
                strips(rw1[s1])
                get_rbuf(p_rf2, rf2, s1)
                get_rbuf(p_rf2, rf2, s1 + 1)
                get_rbuf(p_rw2, rw2, s1)
                get_rbuf(p_rw2, rw2, s1 + 1)
                evict(conv_int("wf1", rf1[s1]), rf2, s1)
                evict(conv_int("ww1", rw1[s1]), rw2, s1)

            s2 = T - 2
            if 0 <= s2 < NSTEPS:
                strips(rf2[s2])
                strips(rw2[s2])
                psf = conv_int("wf2", rf2[s2])
                psw = conv_int("ww2", rw2[s2])
                P = p_act.tile([96, 8, NW], F32, tag="P", name="P")
                nc.scalar.activation(out=P[:], in_=psf[:], func=Relu)
                Et = p_act.tile([96, 8, NW], F32, tag="Et", name="Et")
                nc.scalar.activation(out=Et[:], in_=psw[:], func=Relu)
                E = p_act.tile([96, 8, NW], F32, tag="E", name="E")
                nc.scalar.activation(out=E[:], in_=Et[:], func=Exp)
                EP = p_act.tile([96, 8, NW], F32, tag="EP", name="EP")
                nc.vector.tensor_mul(EP[:], E[:], P[:])
                pse = ps_sm.tile([8, 8, NW], F32, tag="sm", name="pse")
                nc.tensor.matmul(
                    pse[:], _mm_dt(sb["wse"]), _mm_dt(E[:]), start=True, stop=True
                )
                psep = ps_sm.tile([16, 8, NW], F32, tag="sm", name="psep")
                nc.tensor.matmul(
                    psep[:], _mm_dt(sb["wsep"]), _mm_dt(EP[:]), start=True, stop=True
                )
                rcp = p_act.tile([8, 8, NW], F32, tag="rcp", name="rcp")
                nc.vector.reciprocal(out=rcp[:], in_=pse[:])
                psbc = ps_sm.tile([16, 8, NW], F32, tag="sm", name="psbc")
                nc.tensor.matmul(
                    psbc[:], _mm_dt(sb["wbc"]), _mm_dt(rcp[:]), start=True, stop=True
                )
                bcs = p_act.tile([16, 8, NW], F32, tag="bcs", name="bcs")
                nc.scalar.activation(out=bcs[:], in_=psbc[:], func=Copy)
                get_grb(s2)
                get_grb(s2 + 1)
                g = grb[s2]
                nc.vector.tensor_mul(g[0:16, 1:9, :], psep[:], bcs[:])
                sl = s2 % NSLAB
                if sl < NSLAB - 1:
                    nc.vector.tensor_copy(
                        out=grb[s2 + 1][0:16, 0:1, :], in_=g[0:16, 8:9, :]
                    )
                if sl > 0:
                    nc.vector.tensor_copy(
                        out=grb[s2 - 1][0:16, 9:10, :], in_=g[0:16, 1:2, :]
                    )

            s3 = T - 3
            if 0 <= s3 < NSTEPS:
                img, sl = divmod(s3, NSLAB)
                y0 = sl * 8
                g = grb[s3]
                nc.sync.dma_start(out=g[16:18, :, 1:NW], in_=g[14:16, :, 0 : NW - 1])
                nc.sync.dma_start(out=g[18:20, :, 0 : NW - 1], in_=g[0:2, :, 1:NW])
                rt = r0s[s3]
                d = p_d.tile([20, 10, NW], F32, name="d")
                nc.vector.tensor_copy(out=d[:, 1:9, :], in_=rt[64:84, :, :])
                if sl > 0:
                    nc.vector.tensor_copy(
                        out=d[:, 0:1, :], in_=r0s[s3 - 1][64:84, 7:8, :]
                    )
                else:
                    nc.vector.memset(d[:, 0:1, :], 0.0)
                if sl < NSLAB - 1:
                    nc.vector.tensor_copy(
                        out=d[:, 9:10, :], in_=r0s[s3 + 1][64:84, 0:1, :]
                    )
                else:
                    nc.vector.memset(d[:, 9:10, :], 0.0)
                nc.vector.tensor_sub(d[:], d[:], g[:])
                pc = ps_cp.tile([48, 8, NW], F32, tag="cp", name="pc")
                for ky in range(3):
                    nc.tensor.matmul(
                        pc[:],
                        _mm_dt(wky("wchk", ky)),
                        _mm_dt(d[:, ky : ky + 8, :]),
                        start=(ky == 0),
                        stop=False,
                    )
                nc.tensor.matmul(
                    pc[:], _mm_dt(sb["wchm"]), _mm_dt(rt[:]), start=False, stop=True
                )
                stg = p_stg.tile([48, 8, NW], F32, name="stg")
                nc.scalar.activation(out=stg[:], in_=pc[:], func=Copy)
                nc.sync.dma_start(out=out_cp[img, :, y0 : y0 + 8, :], in_=stg[:])
                nc.sync.dma_start(out=out_g[img, :, y0 : y0 + 8, :], in_=g[0:16, 1:9, :])
                for dct in (r0s, rf1, rw1, rf2, rw2, grb):
                    dct.pop(s3 - 2, None)

    nc.compile()
    return nc


_CACHE = {}


def build_core_inputs(inputs):
    mosaic = np.asarray(inputs["mosaic"], np.float32)
    r0_all = build_r0(mosaic)

    stat = {
        "wf0": build_w_l0(np.asarray(inputs["fw0"], np.float32)),
        "ww0": build_w_l0(np.asarray(inputs["ww0"], np.float32)),
        "wf1": build_w_int(np.asarray(inputs["fw1"], np.float32)),
        "wf2": build_w_int(np.asarray(inputs["fw2"], np.float32)),
        "ww1": build_w_int(np.asarray(inputs["ww1"], np.float32)),
        "ww2": build_w_int(np.asarray(inputs["ww2"], np.float32)),
    }
    stat["wse"], stat["wsep"], stat["wbc"] = build_w_sums()
    stat["wchk"], stat["wchm"] = build_w_chroma(np.asarray(inputs["cw0"], np.float32))
    wpack = pack_stationaries(stat)

    in_maps = []
    for c in range(N_CORES):
        in_maps.append(
            {"r0": np.ascontiguousarray(r0_all[c * B_PC : (c + 1) * B_PC]),
             "wpack": wpack}
        )
    return in_maps


def assemble_core_output(mosaic_slice, outs):
    return assemble_output(mosaic_slice, outs["out_cp"], outs["out_g"])


def kernel(mosaic, fw0, fw1, fw2, ww0, ww1, ww2, cw0, _trace=False):
    mosaic = np.asarray(mosaic, np.float32)
    in_maps = build_core_inputs(
        {"mosaic": mosaic, "fw0": fw0, "fw1": fw1, "fw2": fw2,
         "ww0": ww0, "ww1": ww1, "ww2": ww2, "cw0": cw0}
    )

    if "nc" not in _CACHE:
        _CACHE["nc"] = build_program()
    nc = _CACHE["nc"]

    res = run_bass_kernel_spmd(nc, in_maps, list(range(N_CORES)), trace=_trace)
    outs = []
    for c in range(N_CORES):
        outs.append(
            assemble_output(
                mosaic[c * B_PC : (c + 1) * B_PC],
                res.results[c]["out_cp"],
                res.results[c]["out_g"],
            )
        )
    full = np.concatenate(outs, axis=0)
    if _trace:
        return full, res
    return full



# revision 53
# speedup vs baseline: 4.8199x; 4.8199x over previous
"""Trainium2 Bass kernel for BasicQuadRGBModel (quad-Bayer demosaic CNN).

v7 layout — bf16 data path (PSUM accum fp32), HW-calibrated engine split:
  - rq ring tiles [120p, 10 rows, 4 (f1|w1|f2|w2), 64 win] bf16, persistent
    8-slot ring: main block xa=1..8 at partitions (xa-1)*12+ci = [0:96);
    xa=0 strip at [96:108); xa=9 strip at [108:120). Tile s holds f1(s),
    w1(s) (b0 of T=s) and f2(s-4),w2(s-4) (b1 of the same T); one strips
    pass per tile (xa0 on SP queue, xa9 on gpsimd SWDGE) two T later.
    Rows 1..8 hold the slab; halo rows come from neighbor tiles via
    split-window matmuls (ky1 full N=512, ky0/ky2 main N=448 + 1-row halo
    N=64), K=120, M=96.
  - r0 (layer-0 host im2col, ky baked in; ky1-rb block at partitions [0:20)
    so d=rb-g has equal SBUF bases) loaded as [128, 34, 64] tiles covering
    4 slabs + 1-row halo each side.
  - softmax: E = max(exp(psw),1) (exp on Act, max on DVE — NEVER gpsimd:
    7.4us/op there), P = max(psf,0) on DVE, EP on DVE; sums via wse16/wsep
    [96,16] matmuls into one [48]-partition PSUM bank; reciprocal_approx_fast.
  - g4 tiles [20p, 34 rows, 64] covering 4 steps contiguously: 2 halo-row
    copies + 2 col-strip DMAs + 1 out_g DMA per 4 steps.
  - chroma: dS [84p, 8, 64] = ky-stacked d at partitions [0:20),[32:52),
    [64:84) (3 gpsimd subs; gaps zeroed once on persistent tiles) -> ONE
    K=84 matmul; mosaic green_add terms added in the host assembly pass.
  - out_cp staged 4 steps per DMA ([48, 32, 64] bf16); outputs bf16.
  - stages b0=T, b1=T-5, b2=T-10, b3=T-16 to hide DMA issue+sem latency;
    r4 loads prefetched 2 T early; DMA issue spread SP/Act/gpsimd queues.
"""

import os
import sys

sys.path.insert(0, "/opt/trn_rl_repo")

_ABLATE = set(os.environ.get("K_ABLATE", "").split(","))

import numpy as np
import ml_dtypes

import concourse.bass as bass
import concourse.mybir as mybir
import concourse.tile as tile
from concourse import bacc
from concourse.bass_utils import run_bass_kernel_spmd

N_CORES = 8
B_PC = 2
H = W = 512
NW = 64
NSLAB = 64
CH = 12
F32 = mybir.dt.float32
BF16 = mybir.dt.bfloat16
NPBF = ml_dtypes.bfloat16
OUTPUT_NAMES = ["out_cp", "out_g"]


def _rbloc(xa, c):
    if xa == 0:
        return 16 + c
    if xa == 9:
        return 18 + c
    return (xa - 1) * 2 + c


def _rloc(xa, ci):
    if xa == 0:
        return 96 + ci
    if xa == 9:
        return 108 + ci
    return (xa - 1) * 12 + ci


def _r0loc(ky, ci, xa):
    # ky1-rb lives at partitions [0:20) so the d = rb - g TensorTensor has
    # equal SBUF base partitions (hw verifier rule); ky0-ci0/ci3 take [64:84)
    if ky == 0:
        if ci == 0:
            return 64 + xa
        if ci == 3:
            return 74 + xa
        return 20 + _rbloc(xa, ci - 1)
    if ky == 1:
        if ci == 0:
            return 40 + xa
        if ci == 3:
            return 50 + xa
        return _rbloc(xa, ci - 1)
    if ci == 0:
        return 84 + xa
    if ci == 3:
        return 94 + xa
    return 104 + _rbloc(xa, ci - 1)


def build_r0(mosaic):
    B = mosaic.shape[0]
    mp = np.zeros((B, 4, H + 2, W + 2), np.float32)
    mp[:, :, 1 : H + 1, 1 : W + 1] = mosaic
    r0 = np.zeros((B, 128, H, NW), np.float32)
    for ky in range(3):
        for ci in range(4):
            for xa in range(10):
                r0[:, _r0loc(ky, ci, xa)] = mp[:, ci, ky : ky + H, xa : xa + 8 * NW : 8]
    return r0.astype(NPBF)


def build_w_l0(wt):
    W_ = np.zeros((128, 96), np.float32)
    for ky in range(3):
        for ci in range(4):
            for xa in range(10):
                for xo in range(8):
                    kx = xa - xo
                    if 0 <= kx <= 2:
                        for co in range(CH):
                            W_[_r0loc(ky, ci, xa), xo * 12 + co] = wt[co, ci, ky, kx]
    return W_


def build_w_int(wt):
    W_ = np.zeros((3, 120, 96), np.float32)
    for ky in range(3):
        for xa in range(10):
            for xo in range(8):
                kx = xa - xo
                if 0 <= kx <= 2:
                    k = _rloc(xa, 0)
                    W_[ky, k : k + 12, xo * 12 : xo * 12 + 12] = wt[:, :, ky, kx].T
    return W_


def build_w_sums():
    wse = np.zeros((96, 16), np.float32)
    wsep = np.zeros((96, 16), np.float32)
    for xo in range(8):
        for co in range(CH):
            wse[xo * 12 + co, xo * 2] = 1.0
            wse[xo * 12 + co, xo * 2 + 1] = 1.0
            wsep[xo * 12 + co, xo * 2 + (co >= 6)] = 1.0
    return wse, wsep


def build_w_chroma(cw0):
    # wchk[ky] [20, 48]: conv taps on d, plus -d green terms at center ky.
    # The mosaic green_add terms (m0/m3/m1/m2) are added host-side.
    wchk = np.zeros((3, 20, 48), np.float32)
    for ky in range(3):
        for xa in range(10):
            for xo in range(8):
                kx = xa - xo
                if 0 <= kx <= 2:
                    for co in range(6):
                        for d in range(2):
                            wchk[ky, _rbloc(xa, d), xo * 6 + co] = cw0[co, d, ky, kx]
    # green_add = [m0, g1, m3, m0, g0, m3]; g0 = m1 - d0, g1 = m2 - d1
    for xo in range(8):
        wchk[1, _rbloc(xo + 1, 1), xo * 6 + 1] += -1.0
        wchk[1, _rbloc(xo + 1, 0), xo * 6 + 4] += -1.0
    # ky-stacked [84, 48] matching the dS tile blocks at 0:20/32:52/64:84
    wchS = np.zeros((84, 48), np.float32)
    wchS[0:20] = wchk[0]
    wchS[32:52] = wchk[1]
    wchS[64:84] = wchk[2]
    return wchS


def assemble_output(mosaic, cp_dev, g_dev):
    B = mosaic.shape[0]
    cp = np.asarray(cp_dev, np.float32)
    g = np.asarray(g_dev, np.float32)
    cp = cp.reshape(B, 8, 6, H, NW).transpose(0, 2, 3, 4, 1).reshape(B, 6, H, W)
    g = g.reshape(B, 8, 2, H, NW).transpose(0, 2, 3, 4, 1).reshape(B, 2, H, W)
    m = mosaic
    out = np.empty((B, 3, 2 * H, 2 * W), np.float32)
    # chroma_pred = cp + green_add, green_add = [m0, m2, m3, m0, m1, m3]
    # (the -d parts of g0/g1 are already folded into wchS on-device)
    out[:, 0, 0::2, 0::2] = cp[:, 0] + m[:, 0]
    out[:, 0, 0::2, 1::2] = m[:, 1]
    out[:, 0, 1::2, 0::2] = cp[:, 1] + m[:, 2]
    out[:, 0, 1::2, 1::2] = cp[:, 2] + m[:, 3]
    out[:, 1, 0::2, 0::2] = m[:, 0]
    out[:, 1, 0::2, 1::2] = g[:, 0]
    out[:, 1, 1::2, 0::2] = g[:, 1]
    out[:, 1, 1::2, 1::2] = m[:, 3]
    out[:, 2, 0::2, 0::2] = cp[:, 3] + m[:, 0]
    out[:, 2, 0::2, 1::2] = cp[:, 4] + m[:, 1]
    out[:, 2, 1::2, 0::2] = m[:, 2]
    out[:, 2, 1::2, 1::2] = cp[:, 5] + m[:, 3]
    return out


# column offsets inside the packed [128, 1424] stationary tensor
_WOFF = {"wf0": 0, "ww0": 96, "wf1": 192, "wf2": 480, "ww1": 768, "ww2": 1056,
         "wse16": 1344, "wsep": 1360, "wchS": 1376}
_WCOLS = 1424


def pack_stationaries(st):
    wp = np.zeros((128, _WCOLS), np.float32)
    wp[:, 0:96] = st["wf0"]
    wp[:, 96:192] = st["ww0"]
    for nm in ("wf1", "wf2", "ww1", "ww2"):
        o = _WOFF[nm]
        for ky in range(3):
            wp[0:120, o + 96 * ky : o + 96 * (ky + 1)] = st[nm][ky]
    wp[0:96, 1344:1360] = st["wse16"]
    wp[0:96, 1360:1376] = st["wsep"]
    wp[0:84, 1376:1424] = st["wchS"]
    return wp.astype(NPBF)


def build_program():
    from contextlib import ExitStack

    nc = bacc.Bacc(
        "TRN2", target_bir_lowering=False, debug=False, num_devices=N_CORES
    )
    r0 = nc.declare_dram_parameter("r0", [B_PC, 128, H, NW], BF16, isOutput=False)
    wpack = nc.declare_dram_parameter("wpack", [128, _WCOLS], BF16, isOutput=False)
    out_cp = nc.declare_dram_parameter("out_cp", [B_PC, 48, H, NW], BF16, isOutput=True)
    out_g = nc.declare_dram_parameter("out_g", [B_PC, 16, H, NW], BF16, isOutput=True)

    Relu = mybir.ActivationFunctionType.Relu
    Exp = mybir.ActivationFunctionType.Exp
    Copy = mybir.ActivationFunctionType.Copy
    NSTEPS = B_PC * NSLAB
    NT = NSTEPS + 4  # rq tiles (tail tiles carry only f2/w2)

    with tile.TileContext(nc) as tc, ExitStack() as ctx:
        const = ctx.enter_context(tc.tile_pool(name="const", bufs=1))
        p_r4 = ctx.enter_context(tc.tile_pool(name="r4p", bufs=7))
        p_stg = ctx.enter_context(tc.tile_pool(name="stg", bufs=3))
        p_act = ctx.enter_context(tc.tile_pool(name="acts", bufs=3))
        ps_mm = ctx.enter_context(tc.tile_pool(name="psmm", bufs=5, space="PSUM"))
        ps_sm = ctx.enter_context(tc.tile_pool(name="pssm", bufs=1, space="PSUM"))
        ps_cp = ctx.enter_context(tc.tile_pool(name="pscp", bufs=2, space="PSUM"))

        WC = const.tile([128, _WCOLS], BF16, tag="wpack_sb", name="wpack_sb")
        nc.sync.dma_start(out=WC[:], in_=wpack[:])
        sb = {
            "wf0": WC[:, 0:96],
            "ww0": WC[:, 96:192],
            "wse16": WC[0:96, 1344:1360],
            "wsep": WC[0:96, 1360:1376],
            "wchS": WC[0:84, 1376:1424],
        }

        def wky(nm, ky):
            o = _WOFF[nm]
            return WC[0:120, o + 96 * ky : o + 96 * (ky + 1)]

        # persistent rings: logical-tensor reuse keeps one-time edge-column
        # zeros valid (strips DMAs never touch them) and saves per-step memsets
        RQN, G4N, DN = 8, 5, 2
        rq_ring = []
        for i in range(RQN):
            t = const.tile([120, 10, 4, NW], BF16, tag=f"rq{i}", name=f"rq{i}")
            rq_ring.append(t)
            nc.gpsimd.memset(t[96:120, 1:9, :, 0:1], 0.0)
            nc.gpsimd.memset(t[96:120, 1:9, :, 63:64], 0.0)
            # ring slots of head tiles 0..3: zero the not-yet-written f2/w2
            # strip-source regions the first strips DMA will read
            if i < 4:
                nc.gpsimd.memset(t[64:96, 1:9, 2:4, :], 0.0)
                nc.gpsimd.memset(t[0:32, 1:9, 2:4, :], 0.0)
        g4_ring = []
        for i in range(G4N):
            t = const.tile([20, 34, NW], BF16, tag=f"g4_{i}", name=f"g4_{i}")
            g4_ring.append(t)
            nc.vector.memset(t[:, :, 0:1], 0.0)
            nc.vector.memset(t[:, :, 63:64], 0.0)
        d_ring = []
        for i in range(DN):
            t = const.tile([84, 8, NW], BF16, tag=f"ds{i}", name=f"ds{i}")
            d_ring.append(t)
            nc.gpsimd.memset(t[0:32, :, :], 0.0)
            nc.gpsimd.memset(t[32:64, :, :], 0.0)

        r4s, stgs = {}, {}

        def rqt(j):
            return rq_ring[j % RQN]

        def g4t(j):
            return g4_ring[j % G4N]

        def rt_ap(q, a, b):
            """AP of r0-slab rows [a:b) of step q (tile row 0 = slab row -1)."""
            t = r4s[q // 4]
            o = 8 * (q % 4) + 1
            return t[:, o + a : o + b, :]

        _CONV = {"wf1": (0, 0), "ww1": (0, 1), "wf2": (4, 2), "ww2": (4, 3)}

        def conv_int(nm, s):
            """Split-window conv: 5 bf16 matmuls; halo rows come from
            neighbor tiles (tile row x = image row y0+x-1)."""
            ps = ps_mm.tile([96, 8, NW], F32, tag="mm96", name="psc")
            off, b = _CONV[nm]
            t = rqt(s + off)
            sl = s % NSLAB
            nc.tensor.matmul(ps[:], wky(nm, 1), t[:, 1:9, b, :], start=True, stop=False)
            nc.tensor.matmul(
                ps[:, 1:8, :], wky(nm, 0), t[:, 1:8, b, :], start=False, stop=False
            )
            if sl > 0:
                nc.tensor.matmul(
                    ps[:, 0:1, :], wky(nm, 0), rqt(s + off - 1)[:, 8:9, b, :],
                    start=False, stop=False,
                )
            last = sl == NSLAB - 1
            nc.tensor.matmul(
                ps[:, 0:7, :], wky(nm, 2), t[:, 2:9, b, :], start=False, stop=last
            )
            if not last:
                nc.tensor.matmul(
                    ps[:, 7:8, :], wky(nm, 2), rqt(s + off + 1)[:, 1:2, b, :],
                    start=False, stop=True,
                )
            return ps

        def strips(j):
            if "strips" in _ABLATE:
                return
            t = rqt(j)
            nc.sync.dma_start(
                out=t[96:108, 1:9, :, 1:NW], in_=t[84:96, 1:9, :, 0 : NW - 1]
            )
            nc.gpsimd.dma_start(
                out=t[108:120, 1:9, :, 0 : NW - 1], in_=t[0:12, 1:9, :, 1:NW]
            )

        def g4strips(j):
            if "g2strips" in _ABLATE:
                return
            t = g4t(j)
            nc.gpsimd.dma_start(
                out=t[16:18, :, 1:NW], in_=t[14:16, :, 0 : NW - 1]
            )
            nc.gpsimd.dma_start(
                out=t[18:20, :, 0 : NW - 1], in_=t[0:2, :, 1:NW]
            )

        def load_r4(j):
            img, sl4 = divmod(4 * j, NSLAB)
            y0 = sl4 * 8
            r4 = p_r4.tile([128, 34, NW], BF16, name="r4")
            r4s[j] = r4
            if sl4 == 0:
                nc.gpsimd.memset(r4[:, 0:1, :], 0.0)
                nc.scalar.dma_start(out=r4[:, 1:34, :], in_=r0[img, :, 0:33, :])
            elif sl4 == NSLAB - 4:
                nc.scalar.dma_start(
                    out=r4[:, 0:33, :], in_=r0[img, :, y0 - 1 : y0 + 32, :]
                )
                nc.gpsimd.memset(r4[:, 33:34, :], 0.0)
            else:
                nc.scalar.dma_start(
                    out=r4[:], in_=r0[img, :, y0 - 1 : y0 + 33, :]
                )

        for T in range(NSTEPS + 17):
            # strips for tile T-2: its subtiles were all evicted during T-2,
            # so the DMAs' waits are already satisfied when issued
            if 0 <= T - 2 < NT:
                strips(T - 2)

            q0 = T
            if 0 <= q0 < NSTEPS:
                if q0 == 0:
                    load_r4(0)
                # prefetch the 4-slab block two steps ahead of first use
                if (q0 + 2) % 4 == 0 and q0 + 2 < NSTEPS:
                    load_r4((q0 + 2) // 4)
                t1 = rqt(q0)
                psf = ps_mm.tile([96, 8, NW], F32, tag="mm96", name="psf0")
                nc.tensor.matmul(psf[:], sb["wf0"], rt_ap(q0, 0, 8), start=True, stop=True)
                nc.scalar.activation(out=t1[0:96, 1:9, 0, :], in_=psf[:], func=Relu)
                psw = ps_mm.tile([96, 8, NW], F32, tag="mm96", name="psw0")
                nc.tensor.matmul(psw[:], sb["ww0"], rt_ap(q0, 0, 8), start=True, stop=True)
                nc.scalar.activation(out=t1[0:96, 1:9, 1, :], in_=psw[:], func=Relu)

            q1 = T - 5
            if 0 <= q1 < NSTEPS:
                t2 = rqt(q1 + 4)
                psf = conv_int("wf1", q1)
                nc.scalar.activation(out=t2[0:96, 1:9, 2, :], in_=psf[:], func=Relu)
                psw = conv_int("ww1", q1)
                nc.scalar.activation(out=t2[0:96, 1:9, 3, :], in_=psw[:], func=Relu)

            q2 = T - 10
            if 0 <= q2 < NSTEPS:
                sl = q2 % NSLAB
                j4, i4 = divmod(q2, 4)
                og = 8 * i4 + 1
                psf = conv_int("wf2", q2)
                psw = conv_int("ww2", q2)
                P = p_act.tile([96, 8, NW], BF16, tag="P", name="P")
                nc.vector.tensor_scalar_max(P[:], psf[:], 0.0)
                E = p_act.tile([96, 8, NW], BF16, tag="E", name="E")
                nc.scalar.activation(out=E[:], in_=psw[:], func=Exp)
                # relu-before-exp == max(exp, 1)
                nc.vector.tensor_scalar_max(E[:], E[:], 1.0)
                EP = p_act.tile([96, 8, NW], BF16, tag="EP", name="EP")
                nc.vector.tensor_mul(EP[:], E[:], P[:])
                psm = ps_sm.tile([48, 8, NW], F32, tag="sm", name="psm")
                nc.tensor.matmul(psm[0:16, :, :], sb["wse16"], E[:], start=True, stop=True)
                nc.tensor.matmul(psm[32:48, :, :], sb["wsep"], EP[:], start=True, stop=True)
                rcp = p_act.tile([16, 8, NW], F32, tag="rcp", name="rcp")
                nc.vector.reciprocal_approx_fast(out=rcp[:], in_=psm[0:16, :, :])
                g4 = g4t(j4)
                if i4 == 0:
                    if sl == 0:
                        nc.vector.memset(g4[0:16, 0:1, :], 0.0)
                    else:
                        nc.vector.tensor_copy(
                            out=g4[0:16, 0:1, :], in_=g4t(j4 - 1)[0:16, 32:33, :]
                        )
                nc.vector.tensor_mul(g4[0:16, og : og + 8, :], psm[32:48, :, :], rcp[:])
                if i4 == 0 and q2 >= 4:
                    if sl == 0:
                        nc.vector.memset(g4t(j4 - 1)[0:16, 33:34, :], 0.0)
                    else:
                        nc.vector.tensor_copy(
                            out=g4t(j4 - 1)[0:16, 33:34, :], in_=g4[0:16, 1:2, :]
                        )
                    g4strips(j4 - 1)
                if q2 == NSTEPS - 1:
                    nc.vector.memset(g4[0:16, 33:34, :], 0.0)
                    g4strips(j4)
                # out_g for the previous g4 tile (its rows are now final)
                if "outs" not in _ABLATE:
                    jo = None
                    if i4 == 0 and q2 >= 4:
                        jo = j4 - 1
                    elif q2 == NSTEPS - 1:
                        jo = j4
                    if jo is not None:
                        imgo, sl4o = divmod(4 * jo, NSLAB)
                        nc.scalar.dma_start(
                            out=out_g[imgo, :, sl4o * 8 : sl4o * 8 + 32, :],
                            in_=g4t(jo)[0:16, 1:33, :],
                        )

            q3 = T - 16
            if 0 <= q3 < NSTEPS:
                j4, i4 = divmod(q3, 4)
                g4 = g4t(j4)
                r4 = r4s[j4]
                ds = d_ring[q3 % DN]
                o = 8 * i4  # r4/g4 row holding image row y0-1
                for k, p0 in ((0, 0), (1, 32), (2, 64)):
                    nc.gpsimd.tensor_sub(
                        ds[p0 : p0 + 20, :, :],
                        r4[0:20, o + k : o + k + 8, :],
                        g4[0:20, o + k : o + k + 8, :],
                    )
                pc = ps_cp.tile([48, 8, NW], F32, tag="cp", name="pc")
                nc.tensor.matmul(pc[:], sb["wchS"], ds[0:84, :, :], start=True, stop=True)
                if j4 not in stgs:
                    stgs[j4] = p_stg.tile([48, 32, NW], BF16, name="stg")
                nc.scalar.activation(
                    out=stgs[j4][:, 8 * i4 : 8 * i4 + 8, :], in_=pc[:], func=Copy
                )
                # out_cp for the previous stg tile fires one T after it's done
                if "outs" not in _ABLATE:
                    jos = []
                    if i4 == 0 and q3 >= 4:
                        jos.append(j4 - 1)
                    if q3 == NSTEPS - 1:
                        jos.append(j4)
                    for jo in jos:
                        imgo, sl4o = divmod(4 * jo, NSLAB)
                        nc.sync.dma_start(
                            out=out_cp[imgo, :, sl4o * 8 : sl4o * 8 + 32, :],
                            in_=stgs[jo][:],
                        )
                for dct, idx in ((r4s, q3 // 4 - 3), (stgs, q3 // 4 - 2)):
                    dct.pop(idx, None)

    nc.compile()
    return nc


_CACHE = {}


def build_core_inputs(inputs):
    mosaic = np.asarray(inputs["mosaic"], np.float32)
    r0_all = build_r0(mosaic)

    stat = {
        "wf0": build_w_l0(np.asarray(inputs["fw0"], np.float32)),
        "ww0": build_w_l0(np.asarray(inputs["ww0"], np.float32)),
        "wf1": build_w_int(np.asarray(inputs["fw1"], np.float32)),
        "wf2": build_w_int(np.asarray(inputs["fw2"], np.float32)),
        "ww1": build_w_int(np.asarray(inputs["ww1"], np.float32)),
        "ww2": build_w_int(np.asarray(inputs["ww2"], np.float32)),
    }
    stat["wse16"], stat["wsep"] = build_w_sums()
    stat["wchS"] = build_w_chroma(np.asarray(inputs["cw0"], np.float32))
    wpack = pack_stationaries(stat)

    in_maps = []
    for c in range(N_CORES):
        in_maps.append(
            {"r0": np.ascontiguousarray(r0_all[c * B_PC : (c + 1) * B_PC]),
             "wpack": wpack}
        )
    return in_maps


def assemble_core_output(mosaic_slice, outs):
    return assemble_output(mosaic_slice, outs["out_cp"], outs["out_g"])


def kernel(mosaic, fw0, fw1, fw2, ww0, ww1, ww2, cw0, _trace=False):
    mosaic = np.asarray(mosaic, np.float32)
    in_maps = build_core_inputs(
        {"mosaic": mosaic, "fw0": fw0, "fw1": fw1, "fw2": fw2,
         "ww0": ww0, "ww1": ww1, "ww2": ww2, "cw0": cw0}
    )

    if "nc" not in _CACHE:
        _CACHE["nc"] = build_program()
    nc = _CACHE["nc"]

    res = run_bass_kernel_spmd(nc, in_maps, list(range(N_CORES)), trace=_trace)
    outs = []
    for c in range(N_CORES):
        outs.append(
            assemble_output(
                mosaic[c * B_PC : (c + 1) * B_PC],
                res.results[c]["out_cp"],
                res.results[c]["out_g"],
            )
        )
    full = np.concatenate(outs, axis=0)
    if _trace:
        return full, res
    return full


# revision 54
# speedup vs baseline: 5.1181x; 1.0619x over previous
"""Trainium2 Bass kernel for BasicQuadRGBModel (quad-Bayer demosaic CNN).

v7 layout — bf16 data path (PSUM accum fp32), HW-calibrated engine split:
  - rq ring tiles [120p, 10 rows, 4 (f1|w1|f2|w2), 64 win] bf16, persistent
    8-slot ring: main block xa=1..8 at partitions (xa-1)*12+ci = [0:96);
    xa=0 strip at [96:108); xa=9 strip at [108:120). Tile s holds f1(s),
    w1(s) (b0 of T=s) and f2(s-4),w2(s-4) (b1 of the same T); one strips
    pass per tile (xa0 on SP queue, xa9 on gpsimd SWDGE) two T later.
    Rows 1..8 hold the slab; halo rows come from neighbor tiles via
    split-window matmuls (ky1 full N=512, ky0/ky2 main N=448 + 1-row halo
    N=64), K=120, M=96.
  - r0 (layer-0 host im2col, ky baked in; ky1-rb block at partitions [0:20)
    so d=rb-g has equal SBUF bases) loaded as [128, 34, 64] tiles covering
    4 slabs + 1-row halo each side.
  - softmax: E = max(exp(psw),1) (exp on Act, max on DVE — NEVER gpsimd:
    7.4us/op there), P = max(psf,0) on DVE, EP on DVE; sums via wse16/wsep
    [96,16] matmuls into one [48]-partition PSUM bank; reciprocal_approx_fast.
  - g4 tiles [20p, 34 rows, 64] covering 4 steps contiguously: 2 halo-row
    copies + 2 col-strip DMAs + 1 out_g DMA per 4 steps.
  - chroma: dS [84p, 8, 64] = ky-stacked d at partitions [0:20),[32:52),
    [64:84) (3 gpsimd subs; gaps zeroed once on persistent tiles) -> ONE
    K=84 matmul; mosaic green_add terms added in the host assembly pass.
  - out_cp staged 4 steps per DMA ([48, 32, 64] bf16); outputs bf16.
  - stages b0=T, b1=T-5, b2=T-10, b3=T-16 to hide DMA issue+sem latency;
    r4 loads prefetched 2 T early; DMA issue spread SP/Act/gpsimd queues.
"""

import os
import sys

sys.path.insert(0, "/opt/trn_rl_repo")

_ABLATE = set(os.environ.get("K_ABLATE", "").split(","))

import numpy as np
import ml_dtypes

import concourse.bass as bass
import concourse.mybir as mybir
import concourse.tile as tile
from concourse import bacc
from concourse.bass_utils import run_bass_kernel_spmd

N_CORES = 8
B_PC = 2
H = W = 512
NW = 64
NSLAB = 64
CH = 12
F32 = mybir.dt.float32
BF16 = mybir.dt.bfloat16
NPBF = ml_dtypes.bfloat16
OUTPUT_NAMES = ["out_cp", "out_g"]


def _rbloc(xa, c):
    if xa == 0:
        return 16 + c
    if xa == 9:
        return 18 + c
    return (xa - 1) * 2 + c


def _rloc(xa, ci):
    if xa == 0:
        return 96 + ci
    if xa == 9:
        return 108 + ci
    return (xa - 1) * 12 + ci


def _r0loc(ky, ci, xa):
    # ky1-rb lives at partitions [0:20) so the d = rb - g TensorTensor has
    # equal SBUF base partitions (hw verifier rule); ky0-ci0/ci3 take [64:84)
    if ky == 0:
        if ci == 0:
            return 64 + xa
        if ci == 3:
            return 74 + xa
        return 20 + _rbloc(xa, ci - 1)
    if ky == 1:
        if ci == 0:
            return 40 + xa
        if ci == 3:
            return 50 + xa
        return _rbloc(xa, ci - 1)
    if ci == 0:
        return 84 + xa
    if ci == 3:
        return 94 + xa
    return 104 + _rbloc(xa, ci - 1)


def build_r0(mosaic):
    B = mosaic.shape[0]
    mp = np.zeros((B, 4, H + 2, W + 2), np.float32)
    mp[:, :, 1 : H + 1, 1 : W + 1] = mosaic
    r0 = np.zeros((B, 128, H, NW), np.float32)
    for ky in range(3):
        for ci in range(4):
            for xa in range(10):
                r0[:, _r0loc(ky, ci, xa)] = mp[:, ci, ky : ky + H, xa : xa + 8 * NW : 8]
    return r0.astype(NPBF)


def build_w_l0(wt):
    W_ = np.zeros((128, 96), np.float32)
    for ky in range(3):
        for ci in range(4):
            for xa in range(10):
                for xo in range(8):
                    kx = xa - xo
                    if 0 <= kx <= 2:
                        for co in range(CH):
                            W_[_r0loc(ky, ci, xa), xo * 12 + co] = wt[co, ci, ky, kx]
    return W_


def build_w_int(wt):
    W_ = np.zeros((3, 120, 96), np.float32)
    for ky in range(3):
        for xa in range(10):
            for xo in range(8):
                kx = xa - xo
                if 0 <= kx <= 2:
                    k = _rloc(xa, 0)
                    W_[ky, k : k + 12, xo * 12 : xo * 12 + 12] = wt[:, :, ky, kx].T
    return W_


def build_w_sums():
    wse = np.zeros((96, 16), np.float32)
    wsep = np.zeros((96, 16), np.float32)
    for xo in range(8):
        for co in range(CH):
            wse[xo * 12 + co, xo * 2] = 1.0
            wse[xo * 12 + co, xo * 2 + 1] = 1.0
            wsep[xo * 12 + co, xo * 2 + (co >= 6)] = 1.0
    return wse, wsep


def build_w_chroma(cw0):
    # wchk[ky] [20, 48]: conv taps on d, plus -d green terms at center ky.
    # The mosaic green_add terms (m0/m3/m1/m2) are added host-side.
    wchk = np.zeros((3, 20, 48), np.float32)
    for ky in range(3):
        for xa in range(10):
            for xo in range(8):
                kx = xa - xo
                if 0 <= kx <= 2:
                    for co in range(6):
                        for d in range(2):
                            wchk[ky, _rbloc(xa, d), xo * 6 + co] = cw0[co, d, ky, kx]
    # green_add = [m0, g1, m3, m0, g0, m3]; g0 = m1 - d0, g1 = m2 - d1
    for xo in range(8):
        wchk[1, _rbloc(xo + 1, 1), xo * 6 + 1] += -1.0
        wchk[1, _rbloc(xo + 1, 0), xo * 6 + 4] += -1.0
    # ky-stacked [84, 48] matching the dS tile blocks at 0:20/32:52/64:84
    wchS = np.zeros((84, 48), np.float32)
    wchS[0:20] = wchk[0]
    wchS[32:52] = wchk[1]
    wchS[64:84] = wchk[2]
    return wchS


def assemble_output(mosaic, cp_dev, g_dev):
    B = mosaic.shape[0]
    cp = np.asarray(cp_dev, np.float32)
    g = np.asarray(g_dev, np.float32)
    cp = cp.reshape(B, 8, 6, H, NW).transpose(0, 2, 3, 4, 1).reshape(B, 6, H, W)
    g = g.reshape(B, 8, 2, H, NW).transpose(0, 2, 3, 4, 1).reshape(B, 2, H, W)
    m = mosaic
    out = np.empty((B, 3, 2 * H, 2 * W), np.float32)
    # chroma_pred = cp + green_add, green_add = [m0, m2, m3, m0, m1, m3]
    # (the -d parts of g0/g1 are already folded into wchS on-device)
    out[:, 0, 0::2, 0::2] = cp[:, 0] + m[:, 0]
    out[:, 0, 0::2, 1::2] = m[:, 1]
    out[:, 0, 1::2, 0::2] = cp[:, 1] + m[:, 2]
    out[:, 0, 1::2, 1::2] = cp[:, 2] + m[:, 3]
    out[:, 1, 0::2, 0::2] = m[:, 0]
    out[:, 1, 0::2, 1::2] = g[:, 0]
    out[:, 1, 1::2, 0::2] = g[:, 1]
    out[:, 1, 1::2, 1::2] = m[:, 3]
    out[:, 2, 0::2, 0::2] = cp[:, 3] + m[:, 0]
    out[:, 2, 0::2, 1::2] = cp[:, 4] + m[:, 1]
    out[:, 2, 1::2, 0::2] = m[:, 2]
    out[:, 2, 1::2, 1::2] = cp[:, 5] + m[:, 3]
    return out


# column offsets inside the packed [128, 1424] stationary tensor
_WOFF = {"wf0": 0, "ww0": 96, "wf1": 192, "wf2": 480, "ww1": 768, "ww2": 1056,
         "wse16": 1344, "wsep": 1360, "wchS": 1376}
_WCOLS = 1424


def pack_stationaries(st):
    wp = np.zeros((128, _WCOLS), np.float32)
    wp[:, 0:96] = st["wf0"]
    wp[:, 96:192] = st["ww0"]
    for nm in ("wf1", "wf2", "ww1", "ww2"):
        o = _WOFF[nm]
        for ky in range(3):
            wp[0:120, o + 96 * ky : o + 96 * (ky + 1)] = st[nm][ky]
    wp[0:96, 1344:1360] = st["wse16"]
    wp[0:96, 1360:1376] = st["wsep"]
    wp[0:84, 1376:1424] = st["wchS"]
    return wp.astype(NPBF)


def build_program():
    from contextlib import ExitStack

    nc = bacc.Bacc(
        "TRN2", target_bir_lowering=False, debug=False, num_devices=N_CORES
    )
    r0 = nc.declare_dram_parameter("r0", [B_PC, 128, H, NW], BF16, isOutput=False)
    wpack = nc.declare_dram_parameter("wpack", [128, _WCOLS], BF16, isOutput=False)
    out_cp = nc.declare_dram_parameter("out_cp", [B_PC, 48, H, NW], BF16, isOutput=True)
    out_g = nc.declare_dram_parameter("out_g", [B_PC, 16, H, NW], BF16, isOutput=True)

    Relu = mybir.ActivationFunctionType.Relu
    Exp = mybir.ActivationFunctionType.Exp
    Copy = mybir.ActivationFunctionType.Copy
    NSTEPS = B_PC * NSLAB
    NT = NSTEPS + 4  # rq tiles (tail tiles carry only f2/w2)

    with tile.TileContext(nc) as tc, ExitStack() as ctx:
        const = ctx.enter_context(tc.tile_pool(name="const", bufs=1))
        p_r4 = ctx.enter_context(tc.tile_pool(name="r4p", bufs=7))
        p_stg = ctx.enter_context(tc.tile_pool(name="stg", bufs=3))
        p_act = ctx.enter_context(tc.tile_pool(name="acts", bufs=3))
        ps_mm = ctx.enter_context(tc.tile_pool(name="psmm", bufs=5, space="PSUM"))
        ps_sm = ctx.enter_context(tc.tile_pool(name="pssm", bufs=1, space="PSUM"))
        ps_cp = ctx.enter_context(tc.tile_pool(name="pscp", bufs=2, space="PSUM"))

        WC = const.tile([128, _WCOLS], BF16, tag="wpack_sb", name="wpack_sb")
        nc.sync.dma_start(out=WC[:], in_=wpack[:])
        sb = {
            "wf0": WC[:, 0:96],
            "ww0": WC[:, 96:192],
            "wse16": WC[0:96, 1344:1360],
            "wsep": WC[0:96, 1360:1376],
            "wchS": WC[0:84, 1376:1424],
        }

        def wky(nm, ky):
            o = _WOFF[nm]
            return WC[0:120, o + 96 * ky : o + 96 * (ky + 1)]

        # persistent rings: logical-tensor reuse keeps one-time edge-column
        # zeros valid (strips DMAs never touch them) and saves per-step memsets
        RQN, G4N, DN = 8, 5, 3
        rq_ring = []
        for i in range(RQN):
            t = const.tile([120, 10, 4, NW], BF16, tag=f"rq{i}", name=f"rq{i}")
            rq_ring.append(t)
            nc.gpsimd.memset(t[96:120, 1:9, :, 0:1], 0.0)
            nc.gpsimd.memset(t[96:120, 1:9, :, 63:64], 0.0)
            # ring slots of head tiles 0..3: zero the not-yet-written f2/w2
            # strip-source regions the first strips DMA will read
            if i < 4:
                nc.gpsimd.memset(t[64:96, 1:9, 2:4, :], 0.0)
                nc.gpsimd.memset(t[0:32, 1:9, 2:4, :], 0.0)
        g4_ring = []
        for i in range(G4N):
            t = const.tile([20, 34, NW], BF16, tag=f"g4_{i}", name=f"g4_{i}")
            g4_ring.append(t)
            nc.vector.memset(t[:, :, 0:1], 0.0)
            nc.vector.memset(t[:, :, 63:64], 0.0)
        d_ring = []
        for i in range(DN):
            t = const.tile([84, 8, NW], BF16, tag=f"ds{i}", name=f"ds{i}")
            d_ring.append(t)
            nc.gpsimd.memset(t[0:32, :, :], 0.0)
            nc.gpsimd.memset(t[32:64, :, :], 0.0)

        r4s, stgs = {}, {}

        def rqt(j):
            return rq_ring[j % RQN]

        def g4t(j):
            return g4_ring[j % G4N]

        def rt_ap(q, a, b):
            """AP of r0-slab rows [a:b) of step q (tile row 0 = slab row -1)."""
            t = r4s[q // 4]
            o = 8 * (q % 4) + 1
            return t[:, o + a : o + b, :]

        _CONV = {"wf1": (0, 0), "ww1": (0, 1), "wf2": (4, 2), "ww2": (4, 3)}

        def conv_int(nm, s):
            """Split-window conv: 5 bf16 matmuls; halo rows come from
            neighbor tiles (tile row x = image row y0+x-1)."""
            ps = ps_mm.tile([96, 8, NW], F32, tag="mm96", name="psc")
            off, b = _CONV[nm]
            t = rqt(s + off)
            sl = s % NSLAB
            nc.tensor.matmul(ps[:], wky(nm, 1), t[:, 1:9, b, :], start=True, stop=False)
            nc.tensor.matmul(
                ps[:, 1:8, :], wky(nm, 0), t[:, 1:8, b, :], start=False, stop=False
            )
            if sl > 0:
                nc.tensor.matmul(
                    ps[:, 0:1, :], wky(nm, 0), rqt(s + off - 1)[:, 8:9, b, :],
                    start=False, stop=False,
                )
            last = sl == NSLAB - 1
            nc.tensor.matmul(
                ps[:, 0:7, :], wky(nm, 2), t[:, 2:9, b, :], start=False, stop=last
            )
            if not last:
                nc.tensor.matmul(
                    ps[:, 7:8, :], wky(nm, 2), rqt(s + off + 1)[:, 1:2, b, :],
                    start=False, stop=True,
                )
            return ps

        def strips(j):
            if "strips" in _ABLATE:
                return
            t = rqt(j)
            nc.sync.dma_start(
                out=t[96:108, 1:9, :, 1:NW], in_=t[84:96, 1:9, :, 0 : NW - 1]
            )
            nc.gpsimd.dma_start(
                out=t[108:120, 1:9, :, 0 : NW - 1], in_=t[0:12, 1:9, :, 1:NW]
            )

        def g4strips(j):
            if "g2strips" in _ABLATE:
                return
            t = g4t(j)
            nc.gpsimd.dma_start(
                out=t[16:18, :, 1:NW], in_=t[14:16, :, 0 : NW - 1]
            )
            nc.gpsimd.dma_start(
                out=t[18:20, :, 0 : NW - 1], in_=t[0:2, :, 1:NW]
            )

        def load_r4(j):
            img, sl4 = divmod(4 * j, NSLAB)
            y0 = sl4 * 8
            r4 = p_r4.tile([128, 34, NW], BF16, name="r4")
            r4s[j] = r4
            if sl4 == 0:
                nc.gpsimd.memset(r4[:, 0:1, :], 0.0)
                nc.scalar.dma_start(out=r4[:, 1:34, :], in_=r0[img, :, 0:33, :])
            elif sl4 == NSLAB - 4:
                nc.scalar.dma_start(
                    out=r4[:, 0:33, :], in_=r0[img, :, y0 - 1 : y0 + 32, :]
                )
                nc.gpsimd.memset(r4[:, 33:34, :], 0.0)
            else:
                nc.scalar.dma_start(
                    out=r4[:], in_=r0[img, :, y0 - 1 : y0 + 33, :]
                )

        for T in range(NSTEPS + 17):
            # strips for tile T-2: its subtiles were all evicted during T-2,
            # so the DMAs' waits are already satisfied when issued
            if 0 <= T - 2 < NT:
                strips(T - 2)

            q0 = T
            if 0 <= q0 < NSTEPS:
                if q0 == 0:
                    load_r4(0)
                # prefetch the 4-slab block two steps ahead of first use
                if (q0 + 2) % 4 == 0 and q0 + 2 < NSTEPS:
                    load_r4((q0 + 2) // 4)
                t1 = rqt(q0)
                psf = ps_mm.tile([96, 8, NW], F32, tag="mm96", name="psf0")
                nc.tensor.matmul(psf[:], sb["wf0"], rt_ap(q0, 0, 8), start=True, stop=True)
                nc.scalar.activation(out=t1[0:96, 1:9, 0, :], in_=psf[:], func=Relu)
                psw = ps_mm.tile([96, 8, NW], F32, tag="mm96", name="psw0")
                nc.tensor.matmul(psw[:], sb["ww0"], rt_ap(q0, 0, 8), start=True, stop=True)
                nc.scalar.activation(out=t1[0:96, 1:9, 1, :], in_=psw[:], func=Relu)

            q1 = T - 5
            if 0 <= q1 < NSTEPS:
                t2 = rqt(q1 + 4)
                psf = conv_int("wf1", q1)
                nc.scalar.activation(out=t2[0:96, 1:9, 2, :], in_=psf[:], func=Relu)
                psw = conv_int("ww1", q1)
                nc.scalar.activation(out=t2[0:96, 1:9, 3, :], in_=psw[:], func=Relu)

            q2 = T - 10
            if 0 <= q2 < NSTEPS:
                sl = q2 % NSLAB
                j4, i4 = divmod(q2, 4)
                og = 8 * i4 + 1
                psf = conv_int("wf2", q2)
                psw = conv_int("ww2", q2)
                P = p_act.tile([96, 8, NW], BF16, tag="P", name="P")
                nc.vector.tensor_scalar_max(P[:], psf[:], 0.0)
                E = p_act.tile([96, 8, NW], BF16, tag="E", name="E")
                nc.scalar.activation(out=E[:], in_=psw[:], func=Exp)
                # relu-before-exp == max(exp, 1)
                nc.vector.tensor_scalar_max(E[:], E[:], 1.0)
                EP = p_act.tile([96, 8, NW], BF16, tag="EP", name="EP")
                nc.vector.tensor_mul(EP[:], E[:], P[:])
                psm = ps_sm.tile([48, 8, NW], F32, tag="sm", name="psm")
                nc.tensor.matmul(psm[0:16, :, :], sb["wse16"], E[:], start=True, stop=True)
                nc.tensor.matmul(psm[32:48, :, :], sb["wsep"], EP[:], start=True, stop=True)
                rcp = p_act.tile([16, 8, NW], F32, tag="rcp", name="rcp")
                nc.vector.reciprocal_approx_fast(out=rcp[:], in_=psm[0:16, :, :])
                g4 = g4t(j4)
                if i4 == 0:
                    if sl == 0:
                        nc.vector.memset(g4[0:16, 0:1, :], 0.0)
                    else:
                        nc.vector.tensor_copy(
                            out=g4[0:16, 0:1, :], in_=g4t(j4 - 1)[0:16, 32:33, :]
                        )
                nc.vector.tensor_mul(g4[0:16, og : og + 8, :], psm[32:48, :, :], rcp[:])
                if i4 == 0 and q2 >= 4:
                    if sl == 0:
                        nc.vector.memset(g4t(j4 - 1)[0:16, 33:34, :], 0.0)
                    else:
                        nc.vector.tensor_copy(
                            out=g4t(j4 - 1)[0:16, 33:34, :], in_=g4[0:16, 1:2, :]
                        )
                    g4strips(j4 - 1)
                if q2 == NSTEPS - 1:
                    nc.vector.memset(g4[0:16, 33:34, :], 0.0)
                    g4strips(j4)
                # out_g for the previous g4 tile (its rows are now final)
                if "outs" not in _ABLATE:
                    jo = None
                    if i4 == 0 and q2 >= 4:
                        jo = j4 - 1
                    elif q2 == NSTEPS - 1:
                        jo = j4
                    if jo is not None:
                        imgo, sl4o = divmod(4 * jo, NSLAB)
                        nc.scalar.dma_start(
                            out=out_g[imgo, :, sl4o * 8 : sl4o * 8 + 32, :],
                            in_=g4t(jo)[0:16, 1:33, :],
                        )

            q3 = T - 16
            if 0 <= q3 < NSTEPS:
                j4, i4 = divmod(q3, 4)
                g4 = g4t(j4)
                r4 = r4s[j4]
                ds = d_ring[q3 % DN]
                o = 8 * i4  # r4/g4 row holding image row y0-1
                for k, p0, eng in ((0, 0, nc.vector), (1, 32, nc.gpsimd),
                                   (2, 64, nc.vector)):
                    eng.tensor_sub(
                        ds[p0 : p0 + 20, :, :],
                        r4[0:20, o + k : o + k + 8, :],
                        g4[0:20, o + k : o + k + 8, :],
                    )
                pc = ps_cp.tile([48, 8, NW], F32, tag="cp", name="pc")
                nc.tensor.matmul(pc[:], sb["wchS"], ds[0:84, :, :], start=True, stop=True)
                if j4 not in stgs:
                    stgs[j4] = p_stg.tile([48, 32, NW], BF16, name="stg")
                nc.scalar.activation(
                    out=stgs[j4][:, 8 * i4 : 8 * i4 + 8, :], in_=pc[:], func=Copy
                )
                # out_cp for the previous stg tile fires one T after it's done
                if "outs" not in _ABLATE:
                    jos = []
                    if i4 == 0 and q3 >= 4:
                        jos.append(j4 - 1)
                    if q3 == NSTEPS - 1:
                        jos.append(j4)
                    for jo in jos:
                        imgo, sl4o = divmod(4 * jo, NSLAB)
                        nc.sync.dma_start(
                            out=out_cp[imgo, :, sl4o * 8 : sl4o * 8 + 32, :],
                            in_=stgs[jo][:],
                        )
                for dct, idx in ((r4s, q3 // 4 - 3), (stgs, q3 // 4 - 2)):
                    dct.pop(idx, None)

    nc.compile()
    return nc


_CACHE = {}


def build_core_inputs(inputs):
    mosaic = np.asarray(inputs["mosaic"], np.float32)
    r0_all = build_r0(mosaic)

    stat = {
        "wf0": build_w_l0(np.asarray(inputs["fw0"], np.float32)),
        "ww0": build_w_l0(np.asarray(inputs["ww0"], np.float32)),
        "wf1": build_w_int(np.asarray(inputs["fw1"], np.float32)),
        "wf2": build_w_int(np.asarray(inputs["fw2"], np.float32)),
        "ww1": build_w_int(np.asarray(inputs["ww1"], np.float32)),
        "ww2": build_w_int(np.asarray(inputs["ww2"], np.float32)),
    }
    stat["wse16"], stat["wsep"] = build_w_sums()
    stat["wchS"] = build_w_chroma(np.asarray(inputs["cw0"], np.float32))
    wpack = pack_stationaries(stat)

    in_maps = []
    for c in range(N_CORES):
        in_maps.append(
            {"r0": np.ascontiguousarray(r0_all[c * B_PC : (c + 1) * B_PC]),
             "wpack": wpack}
        )
    return in_maps


def assemble_core_output(mosaic_slice, outs):
    return assemble_output(mosaic_slice, outs["out_cp"], outs["out_g"])


def kernel(mosaic, fw0, fw1, fw2, ww0, ww1, ww2, cw0, _trace=False):
    mosaic = np.asarray(mosaic, np.float32)
    in_maps = build_core_inputs(
        {"mosaic": mosaic, "fw0": fw0, "fw1": fw1, "fw2": fw2,
         "ww0": ww0, "ww1": ww1, "ww2": ww2, "cw0": cw0}
    )

    if "nc" not in _CACHE:
        _CACHE["nc"] = build_program()
    nc = _CACHE["nc"]

    res = run_bass_kernel_spmd(nc, in_maps, list(range(N_CORES)), trace=_trace)
    outs = []
    for c in range(N_CORES):
        outs.append(
            assemble_output(
                mosaic[c * B_PC : (c + 1) * B_PC],
                res.results[c]["out_cp"],
                res.results[c]["out_g"],
            )
        )
    full = np.concatenate(outs, axis=0)
    if _trace:
        return full, res
    return full


# revision 55
# speedup vs baseline: 5.1770x; 1.0115x over previous
"""Trainium2 Bass kernel for BasicQuadRGBModel (quad-Bayer demosaic CNN).

v7 layout — bf16 data path (PSUM accum fp32), HW-calibrated engine split:
  - rq ring tiles [120p, 10 rows, 4 (f1|w1|f2|w2), 64 win] bf16, persistent
    8-slot ring: main block xa=1..8 at partitions (xa-1)*12+ci = [0:96);
    xa=0 strip at [96:108); xa=9 strip at [108:120). Tile s holds f1(s),
    w1(s) (b0 of T=s) and f2(s-4),w2(s-4) (b1 of the same T); one strips
    pass per tile (xa0 on SP queue, xa9 on gpsimd SWDGE) two T later.
    Rows 1..8 hold the slab; halo rows come from neighbor tiles via
    split-window matmuls (ky1 full N=512, ky0/ky2 main N=448 + 1-row halo
    N=64), K=120, M=96.
  - r0 (layer-0 host im2col, ky baked in; ky1-rb block at partitions [0:20)
    so d=rb-g has equal SBUF bases) loaded as [128, 34, 64] tiles covering
    4 slabs + 1-row halo each side.
  - softmax: E = max(exp(psw),1) (exp on Act, max on DVE — NEVER gpsimd:
    7.4us/op there), P = max(psf,0) on DVE, EP on DVE; sums via wse16/wsep
    [96,16] matmuls into one [48]-partition PSUM bank; reciprocal_approx_fast.
  - g4 tiles [20p, 34 rows, 64] covering 4 steps contiguously: 2 halo-row
    copies + 2 col-strip DMAs + 1 out_g DMA per 4 steps.
  - chroma: dS [84p, 8, 64] = ky-stacked d at partitions [0:20),[32:52),
    [64:84) (3 gpsimd subs; gaps zeroed once on persistent tiles) -> ONE
    K=84 matmul; mosaic green_add terms added in the host assembly pass.
  - out_cp staged 4 steps per DMA ([48, 32, 64] bf16); outputs bf16.
  - stages b0=T, b1=T-5, b2=T-10, b3=T-16 to hide DMA issue+sem latency;
    r4 loads prefetched 2 T early; DMA issue spread SP/Act/gpsimd queues.
"""

import os
import sys

sys.path.insert(0, "/opt/trn_rl_repo")

_ABLATE = set(os.environ.get("K_ABLATE", "").split(","))

import numpy as np
import ml_dtypes

import concourse.bass as bass
import concourse.mybir as mybir
import concourse.tile as tile
from concourse import bacc
from concourse.bass_utils import run_bass_kernel_spmd

N_CORES = 8
B_PC = 2
H = W = 512
NW = 64
NSLAB = 64
CH = 12
F32 = mybir.dt.float32
BF16 = mybir.dt.bfloat16
NPBF = ml_dtypes.bfloat16
OUTPUT_NAMES = ["out_cp", "out_g"]


def _rbloc(xa, c):
    if xa == 0:
        return 16 + c
    if xa == 9:
        return 18 + c
    return (xa - 1) * 2 + c


def _rloc(xa, ci):
    if xa == 0:
        return 96 + ci
    if xa == 9:
        return 108 + ci
    return (xa - 1) * 12 + ci


def _r0loc(ky, ci, xa):
    # ky1-rb lives at partitions [0:20) so the d = rb - g TensorTensor has
    # equal SBUF base partitions (hw verifier rule); ky0-ci0/ci3 take [64:84)
    if ky == 0:
        if ci == 0:
            return 64 + xa
        if ci == 3:
            return 74 + xa
        return 20 + _rbloc(xa, ci - 1)
    if ky == 1:
        if ci == 0:
            return 40 + xa
        if ci == 3:
            return 50 + xa
        return _rbloc(xa, ci - 1)
    if ci == 0:
        return 84 + xa
    if ci == 3:
        return 94 + xa
    return 104 + _rbloc(xa, ci - 1)


def build_r0(mosaic):
    B = mosaic.shape[0]
    mp = np.zeros((B, 4, H + 2, W + 2), np.float32)
    mp[:, :, 1 : H + 1, 1 : W + 1] = mosaic
    r0 = np.zeros((B, 128, H, NW), np.float32)
    for ky in range(3):
        for ci in range(4):
            for xa in range(10):
                r0[:, _r0loc(ky, ci, xa)] = mp[:, ci, ky : ky + H, xa : xa + 8 * NW : 8]
    return r0.astype(NPBF)


def build_w_l0(wt):
    W_ = np.zeros((128, 96), np.float32)
    for ky in range(3):
        for ci in range(4):
            for xa in range(10):
                for xo in range(8):
                    kx = xa - xo
                    if 0 <= kx <= 2:
                        for co in range(CH):
                            W_[_r0loc(ky, ci, xa), xo * 12 + co] = wt[co, ci, ky, kx]
    return W_


def build_w_int(wt):
    W_ = np.zeros((3, 120, 96), np.float32)
    for ky in range(3):
        for xa in range(10):
            for xo in range(8):
                kx = xa - xo
                if 0 <= kx <= 2:
                    k = _rloc(xa, 0)
                    W_[ky, k : k + 12, xo * 12 : xo * 12 + 12] = wt[:, :, ky, kx].T
    return W_


def build_w_sums():
    wse = np.zeros((96, 16), np.float32)
    wsep = np.zeros((96, 16), np.float32)
    for xo in range(8):
        for co in range(CH):
            wse[xo * 12 + co, xo * 2] = 1.0
            wse[xo * 12 + co, xo * 2 + 1] = 1.0
            wsep[xo * 12 + co, xo * 2 + (co >= 6)] = 1.0
    return wse, wsep


def build_w_chroma(cw0):
    # wchk[ky] [20, 48]: conv taps on d, plus -d green terms at center ky.
    # The mosaic green_add terms (m0/m3/m1/m2) are added host-side.
    wchk = np.zeros((3, 20, 48), np.float32)
    for ky in range(3):
        for xa in range(10):
            for xo in range(8):
                kx = xa - xo
                if 0 <= kx <= 2:
                    for co in range(6):
                        for d in range(2):
                            wchk[ky, _rbloc(xa, d), xo * 6 + co] = cw0[co, d, ky, kx]
    # green_add = [m0, g1, m3, m0, g0, m3]; g0 = m1 - d0, g1 = m2 - d1
    for xo in range(8):
        wchk[1, _rbloc(xo + 1, 1), xo * 6 + 1] += -1.0
        wchk[1, _rbloc(xo + 1, 0), xo * 6 + 4] += -1.0
    # ky-stacked [84, 48] matching the dS tile blocks at 0:20/32:52/64:84
    wchS = np.zeros((84, 48), np.float32)
    wchS[0:20] = wchk[0]
    wchS[32:52] = wchk[1]
    wchS[64:84] = wchk[2]
    return wchS


def assemble_output(mosaic, cp_dev, g_dev):
    B = mosaic.shape[0]
    cp = np.asarray(cp_dev, np.float32)
    g = np.asarray(g_dev, np.float32)
    cp = cp.reshape(B, 8, 6, H, NW).transpose(0, 2, 3, 4, 1).reshape(B, 6, H, W)
    g = g.reshape(B, 8, 2, H, NW).transpose(0, 2, 3, 4, 1).reshape(B, 2, H, W)
    m = mosaic
    out = np.empty((B, 3, 2 * H, 2 * W), np.float32)
    # chroma_pred = cp + green_add, green_add = [m0, m2, m3, m0, m1, m3]
    # (the -d parts of g0/g1 are already folded into wchS on-device)
    out[:, 0, 0::2, 0::2] = cp[:, 0] + m[:, 0]
    out[:, 0, 0::2, 1::2] = m[:, 1]
    out[:, 0, 1::2, 0::2] = cp[:, 1] + m[:, 2]
    out[:, 0, 1::2, 1::2] = cp[:, 2] + m[:, 3]
    out[:, 1, 0::2, 0::2] = m[:, 0]
    out[:, 1, 0::2, 1::2] = g[:, 0]
    out[:, 1, 1::2, 0::2] = g[:, 1]
    out[:, 1, 1::2, 1::2] = m[:, 3]
    out[:, 2, 0::2, 0::2] = cp[:, 3] + m[:, 0]
    out[:, 2, 0::2, 1::2] = cp[:, 4] + m[:, 1]
    out[:, 2, 1::2, 0::2] = m[:, 2]
    out[:, 2, 1::2, 1::2] = cp[:, 5] + m[:, 3]
    return out


# column offsets inside the packed [128, 1424] stationary tensor
_WOFF = {"wf0": 0, "ww0": 96, "wf1": 192, "wf2": 480, "ww1": 768, "ww2": 1056,
         "wse16": 1344, "wsep": 1360, "wchS": 1376}
_WCOLS = 1424


def pack_stationaries(st):
    wp = np.zeros((128, _WCOLS), np.float32)
    wp[:, 0:96] = st["wf0"]
    wp[:, 96:192] = st["ww0"]
    for nm in ("wf1", "wf2", "ww1", "ww2"):
        o = _WOFF[nm]
        for ky in range(3):
            wp[0:120, o + 96 * ky : o + 96 * (ky + 1)] = st[nm][ky]
    wp[0:96, 1344:1360] = st["wse16"]
    wp[0:96, 1360:1376] = st["wsep"]
    wp[0:84, 1376:1424] = st["wchS"]
    return wp.astype(NPBF)


def build_program():
    from contextlib import ExitStack

    nc = bacc.Bacc(
        "TRN2", target_bir_lowering=False, debug=False, num_devices=N_CORES
    )
    r0 = nc.declare_dram_parameter("r0", [B_PC, 128, H, NW], BF16, isOutput=False)
    wpack = nc.declare_dram_parameter("wpack", [128, _WCOLS], BF16, isOutput=False)
    out_cp = nc.declare_dram_parameter("out_cp", [B_PC, 48, H, NW], BF16, isOutput=True)
    out_g = nc.declare_dram_parameter("out_g", [B_PC, 16, H, NW], BF16, isOutput=True)

    Relu = mybir.ActivationFunctionType.Relu
    Exp = mybir.ActivationFunctionType.Exp
    Copy = mybir.ActivationFunctionType.Copy
    NSTEPS = B_PC * NSLAB
    NT = NSTEPS + 4  # rq tiles (tail tiles carry only f2/w2)

    with tile.TileContext(nc) as tc, ExitStack() as ctx:
        const = ctx.enter_context(tc.tile_pool(name="const", bufs=1))
        p_r4 = ctx.enter_context(tc.tile_pool(name="r4p", bufs=7))
        p_stg = ctx.enter_context(tc.tile_pool(name="stg", bufs=3))
        p_act = ctx.enter_context(tc.tile_pool(name="acts", bufs=3))
        ps_mm = ctx.enter_context(tc.tile_pool(name="psmm", bufs=5, space="PSUM"))
        ps_sm = ctx.enter_context(tc.tile_pool(name="pssm", bufs=1, space="PSUM"))
        ps_cp = ctx.enter_context(tc.tile_pool(name="pscp", bufs=2, space="PSUM"))

        WC = const.tile([128, _WCOLS], BF16, tag="wpack_sb", name="wpack_sb")
        nc.sync.dma_start(out=WC[:], in_=wpack[:])
        sb = {
            "wf0": WC[:, 0:96],
            "ww0": WC[:, 96:192],
            "wse16": WC[0:96, 1344:1360],
            "wsep": WC[0:96, 1360:1376],
            "wchS": WC[0:84, 1376:1424],
        }

        def wky(nm, ky):
            o = _WOFF[nm]
            return WC[0:120, o + 96 * ky : o + 96 * (ky + 1)]

        # persistent rings: logical-tensor reuse keeps one-time edge-column
        # zeros valid (strips DMAs never touch them) and saves per-step memsets
        RQN, G4N, DN = 8, 5, 3
        rq_ring = []
        for i in range(RQN):
            t = const.tile([120, 10, 4, NW], BF16, tag=f"rq{i}", name=f"rq{i}")
            rq_ring.append(t)
            nc.gpsimd.memset(t[96:120, 1:9, :, 0:1], 0.0)
            nc.gpsimd.memset(t[96:120, 1:9, :, 63:64], 0.0)
            # ring slots of head tiles 0..3: zero the not-yet-written f2/w2
            # strip-source regions the first strips DMA will read
            if i < 4:
                nc.gpsimd.memset(t[64:96, 1:9, 2:4, :], 0.0)
                nc.gpsimd.memset(t[0:32, 1:9, 2:4, :], 0.0)
        g4_ring = []
        for i in range(G4N):
            t = const.tile([20, 34, NW], BF16, tag=f"g4_{i}", name=f"g4_{i}")
            g4_ring.append(t)
            nc.vector.memset(t[:, :, 0:1], 0.0)
            nc.vector.memset(t[:, :, 63:64], 0.0)
        d_ring = []
        for i in range(DN):
            t = const.tile([84, 8, NW], BF16, tag=f"ds{i}", name=f"ds{i}")
            d_ring.append(t)
            nc.gpsimd.memset(t[0:32, :, :], 0.0)
            nc.gpsimd.memset(t[32:64, :, :], 0.0)

        r4s, stgs, acts = {}, {}, {}

        def rqt(j):
            return rq_ring[j % RQN]

        def g4t(j):
            return g4_ring[j % G4N]

        def rt_ap(q, a, b):
            """AP of r0-slab rows [a:b) of step q (tile row 0 = slab row -1)."""
            t = r4s[q // 4]
            o = 8 * (q % 4) + 1
            return t[:, o + a : o + b, :]

        _CONV = {"wf1": (0, 0), "ww1": (0, 1), "wf2": (4, 2), "ww2": (4, 3)}

        def conv_int(nm, s):
            """Split-window conv: 5 bf16 matmuls; halo rows come from
            neighbor tiles (tile row x = image row y0+x-1)."""
            ps = ps_mm.tile([96, 8, NW], F32, tag="mm96", name="psc")
            off, b = _CONV[nm]
            t = rqt(s + off)
            sl = s % NSLAB
            nc.tensor.matmul(ps[:], wky(nm, 1), t[:, 1:9, b, :], start=True, stop=False)
            nc.tensor.matmul(
                ps[:, 1:8, :], wky(nm, 0), t[:, 1:8, b, :], start=False, stop=False
            )
            if sl > 0:
                nc.tensor.matmul(
                    ps[:, 0:1, :], wky(nm, 0), rqt(s + off - 1)[:, 8:9, b, :],
                    start=False, stop=False,
                )
            last = sl == NSLAB - 1
            nc.tensor.matmul(
                ps[:, 0:7, :], wky(nm, 2), t[:, 2:9, b, :], start=False, stop=last
            )
            if not last:
                nc.tensor.matmul(
                    ps[:, 7:8, :], wky(nm, 2), rqt(s + off + 1)[:, 1:2, b, :],
                    start=False, stop=True,
                )
            return ps

        def strips(j):
            if "strips" in _ABLATE:
                return
            t = rqt(j)
            nc.sync.dma_start(
                out=t[96:108, 1:9, :, 1:NW], in_=t[84:96, 1:9, :, 0 : NW - 1]
            )
            nc.gpsimd.dma_start(
                out=t[108:120, 1:9, :, 0 : NW - 1], in_=t[0:12, 1:9, :, 1:NW]
            )

        def g4strips(j):
            if "g2strips" in _ABLATE:
                return
            t = g4t(j)
            nc.gpsimd.dma_start(
                out=t[16:18, :, 1:NW], in_=t[14:16, :, 0 : NW - 1]
            )
            nc.gpsimd.dma_start(
                out=t[18:20, :, 0 : NW - 1], in_=t[0:2, :, 1:NW]
            )

        def load_r4(j):
            img, sl4 = divmod(4 * j, NSLAB)
            y0 = sl4 * 8
            r4 = p_r4.tile([128, 34, NW], BF16, name="r4")
            r4s[j] = r4
            if sl4 == 0:
                nc.gpsimd.memset(r4[:, 0:1, :], 0.0)
                nc.scalar.dma_start(out=r4[:, 1:34, :], in_=r0[img, :, 0:33, :])
            elif sl4 == NSLAB - 4:
                nc.scalar.dma_start(
                    out=r4[:, 0:33, :], in_=r0[img, :, y0 - 1 : y0 + 32, :]
                )
                nc.gpsimd.memset(r4[:, 33:34, :], 0.0)
            else:
                nc.scalar.dma_start(
                    out=r4[:], in_=r0[img, :, y0 - 1 : y0 + 33, :]
                )

        for T in range(NSTEPS + 17):
            # strips for tile T-2: its subtiles were all evicted during T-2,
            # so the DMAs' waits are already satisfied when issued
            if 0 <= T - 2 < NT:
                strips(T - 2)

            q0 = T
            if 0 <= q0 < NSTEPS:
                if q0 == 0:
                    load_r4(0)
                # prefetch the 4-slab block two steps ahead of first use
                if (q0 + 2) % 4 == 0 and q0 + 2 < NSTEPS:
                    load_r4((q0 + 2) // 4)
                t1 = rqt(q0)
                psf = ps_mm.tile([96, 8, NW], F32, tag="mm96", name="psf0")
                nc.tensor.matmul(psf[:], sb["wf0"], rt_ap(q0, 0, 8), start=True, stop=True)
                nc.scalar.activation(out=t1[0:96, 1:9, 0, :], in_=psf[:], func=Relu)
                psw = ps_mm.tile([96, 8, NW], F32, tag="mm96", name="psw0")
                nc.tensor.matmul(psw[:], sb["ww0"], rt_ap(q0, 0, 8), start=True, stop=True)
                nc.scalar.activation(out=t1[0:96, 1:9, 1, :], in_=psw[:], func=Relu)

            q1 = T - 5
            if 0 <= q1 < NSTEPS:
                t2 = rqt(q1 + 4)
                psf = conv_int("wf1", q1)
                nc.scalar.activation(out=t2[0:96, 1:9, 2, :], in_=psf[:], func=Relu)
                psw = conv_int("ww1", q1)
                nc.scalar.activation(out=t2[0:96, 1:9, 3, :], in_=psw[:], func=Relu)

            # b2a: final convs + PSUM evictions (E/P/EP ready a full T
            # before the PE-side sums consume them — no PE inline waits)
            q2a = T - 9
            if 0 <= q2a < NSTEPS:
                psf = conv_int("wf2", q2a)
                psw = conv_int("ww2", q2a)
                P = p_act.tile([96, 8, NW], BF16, tag="P", name="P")
                nc.vector.tensor_scalar_max(P[:], psf[:], 0.0)
                E = p_act.tile([96, 8, NW], BF16, tag="E", name="E")
                nc.scalar.activation(out=E[:], in_=psw[:], func=Exp)
                # relu-before-exp == max(exp, 1)
                nc.vector.tensor_scalar_max(E[:], E[:], 1.0)
                EP = p_act.tile([96, 8, NW], BF16, tag="EP", name="EP")
                nc.vector.tensor_mul(EP[:], E[:], P[:])
                acts[q2a] = (E, EP)

            q2 = T - 10
            if 0 <= q2 < NSTEPS:
                sl = q2 % NSLAB
                j4, i4 = divmod(q2, 4)
                og = 8 * i4 + 1
                E, EP = acts.pop(q2)
                psm = ps_sm.tile([48, 8, NW], F32, tag="sm", name="psm")
                nc.tensor.matmul(psm[0:16, :, :], sb["wse16"], E[:], start=True, stop=True)
                nc.tensor.matmul(psm[32:48, :, :], sb["wsep"], EP[:], start=True, stop=True)
                rcp = p_act.tile([16, 8, NW], F32, tag="rcp", name="rcp")
                nc.vector.reciprocal_approx_fast(out=rcp[:], in_=psm[0:16, :, :])
                g4 = g4t(j4)
                if i4 == 0:
                    if sl == 0:
                        nc.vector.memset(g4[0:16, 0:1, :], 0.0)
                    else:
                        nc.vector.tensor_copy(
                            out=g4[0:16, 0:1, :], in_=g4t(j4 - 1)[0:16, 32:33, :]
                        )
                nc.vector.tensor_mul(g4[0:16, og : og + 8, :], psm[32:48, :, :], rcp[:])
                if i4 == 0 and q2 >= 4:
                    if sl == 0:
                        nc.vector.memset(g4t(j4 - 1)[0:16, 33:34, :], 0.0)
                    else:
                        nc.vector.tensor_copy(
                            out=g4t(j4 - 1)[0:16, 33:34, :], in_=g4[0:16, 1:2, :]
                        )
                    g4strips(j4 - 1)
                if q2 == NSTEPS - 1:
                    nc.vector.memset(g4[0:16, 33:34, :], 0.0)
                    g4strips(j4)
                # out_g for the previous g4 tile (its rows are now final)
                if "outs" not in _ABLATE:
                    jo = None
                    if i4 == 0 and q2 >= 4:
                        jo = j4 - 1
                    elif q2 == NSTEPS - 1:
                        jo = j4
                    if jo is not None:
                        imgo, sl4o = divmod(4 * jo, NSLAB)
                        nc.scalar.dma_start(
                            out=out_g[imgo, :, sl4o * 8 : sl4o * 8 + 32, :],
                            in_=g4t(jo)[0:16, 1:33, :],
                        )

            # b3a: dS subs one T before the chroma matmul consumes them
            q3a = T - 15
            if 0 <= q3a < NSTEPS:
                j4, i4 = divmod(q3a, 4)
                g4 = g4t(j4)
                r4 = r4s[j4]
                ds = d_ring[q3a % DN]
                o = 8 * i4  # r4/g4 row holding image row y0-1
                for k, p0, eng in ((0, 0, nc.vector), (1, 32, nc.gpsimd),
                                   (2, 64, nc.vector)):
                    eng.tensor_sub(
                        ds[p0 : p0 + 20, :, :],
                        r4[0:20, o + k : o + k + 8, :],
                        g4[0:20, o + k : o + k + 8, :],
                    )

            q3 = T - 16
            if 0 <= q3 < NSTEPS:
                j4, i4 = divmod(q3, 4)
                ds = d_ring[q3 % DN]
                pc = ps_cp.tile([48, 8, NW], F32, tag="cp", name="pc")
                nc.tensor.matmul(pc[:], sb["wchS"], ds[0:84, :, :], start=True, stop=True)
                if j4 not in stgs:
                    stgs[j4] = p_stg.tile([48, 32, NW], BF16, name="stg")
                nc.scalar.activation(
                    out=stgs[j4][:, 8 * i4 : 8 * i4 + 8, :], in_=pc[:], func=Copy
                )
                # out_cp for the previous stg tile fires one T after it's done
                if "outs" not in _ABLATE:
                    jos = []
                    if i4 == 0 and q3 >= 4:
                        jos.append(j4 - 1)
                    if q3 == NSTEPS - 1:
                        jos.append(j4)
                    for jo in jos:
                        imgo, sl4o = divmod(4 * jo, NSLAB)
                        nc.sync.dma_start(
                            out=out_cp[imgo, :, sl4o * 8 : sl4o * 8 + 32, :],
                            in_=stgs[jo][:],
                        )
                for dct, idx in ((r4s, q3 // 4 - 3), (stgs, q3 // 4 - 2)):
                    dct.pop(idx, None)

    nc.compile()
    return nc


_CACHE = {}


def build_core_inputs(inputs):
    mosaic = np.asarray(inputs["mosaic"], np.float32)
    r0_all = build_r0(mosaic)

    stat = {
        "wf0": build_w_l0(np.asarray(inputs["fw0"], np.float32)),
        "ww0": build_w_l0(np.asarray(inputs["ww0"], np.float32)),
        "wf1": build_w_int(np.asarray(inputs["fw1"], np.float32)),
        "wf2": build_w_int(np.asarray(inputs["fw2"], np.float32)),
        "ww1": build_w_int(np.asarray(inputs["ww1"], np.float32)),
        "ww2": build_w_int(np.asarray(inputs["ww2"], np.float32)),
    }
    stat["wse16"], stat["wsep"] = build_w_sums()
    stat["wchS"] = build_w_chroma(np.asarray(inputs["cw0"], np.float32))
    wpack = pack_stationaries(stat)

    in_maps = []
    for c in range(N_CORES):
        in_maps.append(
            {"r0": np.ascontiguousarray(r0_all[c * B_PC : (c + 1) * B_PC]),
             "wpack": wpack}
        )
    return in_maps


def assemble_core_output(mosaic_slice, outs):
    return assemble_output(mosaic_slice, outs["out_cp"], outs["out_g"])


def kernel(mosaic, fw0, fw1, fw2, ww0, ww1, ww2, cw0, _trace=False):
    mosaic = np.asarray(mosaic, np.float32)
    in_maps = build_core_inputs(
        {"mosaic": mosaic, "fw0": fw0, "fw1": fw1, "fw2": fw2,
         "ww0": ww0, "ww1": ww1, "ww2": ww2, "cw0": cw0}
    )

    if "nc" not in _CACHE:
        _CACHE["nc"] = build_program()
    nc = _CACHE["nc"]

    res = run_bass_kernel_spmd(nc, in_maps, list(range(N_CORES)), trace=_trace)
    outs = []
    for c in range(N_CORES):
        outs.append(
            assemble_output(
                mosaic[c * B_PC : (c + 1) * B_PC],
                res.results[c]["out_cp"],
                res.results[c]["out_g"],
            )
        )
    full = np.concatenate(outs, axis=0)
    if _trace:
        return full, res
    return full
